# revision 8
# baseline (speedup 1.0000x reference)
"""Trainium2 Bass kernel for nn_AttentionBlock (AdaGroupNorm + self-attention).

Full-input contract: kernel(**inputs) takes the unsharded inputs and returns
the full [4, 256, 64, 64] output. Internally shards across 8 NeuronCores:
core c handles batch b = c // 2, token half h = c % 2 (2048 of 4096 tokens).
Each core receives x[b] channel-major [256, 4096] with its own 2048 q-tokens
rotated to the front (GroupNorm stats, k/v and softmax are invariant to token
permutation), computes attention rows for those tokens against all 4096 k/v,
and returns a [256, 2048] slab; the host concatenates.

v2 design (ACT-bound): the scalar engine only runs the 64 softmax Exp
instructions (the hard floor: 8.4M elements/core at 1 elem/lane/cycle).
  - GroupNorm: bn_stats per channel on DVE; group pooling / broadcast across
    partitions via tiny matmuls with host 0/1 group matrices; rstd via Newton
    rsqrt on DVE. AdaGN apply (y = x*A + B) on GPSIMD (SBUF-only op).
  - q/k projections (bf16) evacuate PSUM through DVE tensor_scalar as
    fp8e4m3 scaled 1/4 so S = q8 . k8 equals logits/sqrt(C); v token-major
    fp8 via DVE copy; v-bias is folded into the proj bias on the host
    (normalization makes it a per-channel constant).
  - Attention (QC=512 q-chunks): S pairs [128, 2, 512] via fp8 DoubleRow
    matmuls; ONE Exp per k-tile pair (N=1024) writing the DoubleRow-packed
    fp8 P pair; attn@v fp8 DoubleRow; softmax denominator accumulated on PE
    by an all-ones fp8 matmul over the same P pairs into a persistent
    [128, 512] PSUM tile (denominator replicated across partitions — no
    DVE/GPSIMD partial sums and no broadcast step at all).
  - Denominator reciprocal on DVE; applied once to the attn@v output.
  - proj (bf16) + bias + residual (x read straight from its SBUF tile).
Persistent tiles are double-buffered so consecutive reps overlap fully.
"""

import sys

import numpy as np

for _p in ("/opt/trn_rl_repo",):
    if _p not in sys.path:
        sys.path.insert(0, _p)

import concourse.bass as bass
import concourse.bacc as bacc
import concourse.mybir as mybir
import concourse.tile as tile
from concourse.bass_utils import run_bass_kernel_spmd

F32 = mybir.dt.float32
F32R = mybir.dt.float32r
BF16 = mybir.dt.bfloat16
FP8 = mybir.dt.float8e4
AF = mybir.ActivationFunctionType
OP = mybir.AluOpType
DR = mybir.MatmulPerfMode.DoubleRow

B, C, HW = 4, 256, 4096
TQ = HW // 2          # q tokens per core
G = 32                # num groups
GS = C // G           # channels per group
COND = 512
EPS = 1e-5
N_CORES = 8

CT = C // 128         # channel tiles (2)
KT = HW // 128        # k-token tiles (32)
NP = KT // 2          # k-tile pairs (16)
QC = 512              # q-chunk width in attention
NQC = TQ // QC        # q chunks (4)


def _r(ap):
    """View an fp32 AP as float32r for full-rate PE matmuls."""
    if ap.dtype == F32:
        return ap.bitcast(F32R)
    return ap


def build_nc(reps: int = 1) -> bass.Bass:
    nc = bacc.Bacc()

    xt_d = nc.dram_tensor("xt", [C, HW], F32, kind="ExternalInput")
    cond_d = nc.dram_tensor("cond_t", [128, 4], F32, kind="ExternalInput")
    linw_d = nc.dram_tensor("lin_w", [COND, 2 * C], F32, kind="ExternalInput")
    linbT_d = nc.dram_tensor("lin_bT", [128, 4], F32, kind="ExternalInput")
    qkvw_d = nc.dram_tensor("qkv_w", [C, 3 * C], F32, kind="ExternalInput")
    qkvbT_d = nc.dram_tensor("qkv_bT", [128, 6], F32, kind="ExternalInput")
    projw_d = nc.dram_tensor("proj_w", [C, C], F32, kind="ExternalInput")
    projbT_d = nc.dram_tensor("proj_bT", [128, 2], F32, kind="ExternalInput")
    gpool_d = nc.dram_tensor("gpool", [128, 16], F32, kind="ExternalInput")
    gbcast_d = nc.dram_tensor("gbcast", [16, 128], F32, kind="ExternalInput")
    out_d = nc.dram_tensor("out", [C, TQ], F32, kind="ExternalOutput")
    sbsc_d = nc.dram_tensor("sb_scratch", [4, 128], F32)

    with tile.TileContext(nc) as tc:
        with (
            nc.allow_low_precision(reason="float32r/fp8 rounding for PE matmuls"),
            tc.tile_pool(name="persist", bufs=1) as pp,
            tc.tile_pool(name="wp", bufs=1) as wp,
            tc.tile_pool(name="sb_p", bufs=3) as sp,   # fp8 P pair tiles
            tc.tile_pool(name="sb_r", bufs=2) as rp,   # rb broadcast tiles
            tc.tile_pool(name="sb_w", bufs=2) as sw,   # misc working tiles
            tc.tile_pool(name="sb_s", bufs=2) as ss,   # tiny scalars
            tc.tile_pool(name="ps", bufs=1, space="PSUM") as ps,
        ):
            # ---- persistent SBUF (bufs=2 so reps overlap) ----
            xt = [pp.tile([128, HW], F32, tag=f"xt{t}", name=f"xt{t}", bufs=2)
                  for t in range(CT)]
            hh = [pp.tile([128, HW], BF16, tag=f"hh{t}", name=f"hh{t}", bufs=2)
                  for t in range(CT)]
            kT8 = pp.tile([128, CT, HW], FP8, tag="kT8", name="kT8", bufs=2)
            qT8 = pp.tile([128, CT, TQ], FP8, tag="qT8", name="qT8", bufs=2)
            vtok = pp.tile([128, KT, C], FP8, tag="vtok", name="vtok", bufs=2)
            oT = [pp.tile([128, TQ], BF16, tag=f"oT{t}", name=f"oT{t}", bufs=2)
                  for t in range(CT)]

            # ---- weights / constants ----
            condt = wp.tile([128, 4], F32R, name="condt")
            nc.gpsimd.dma_start(out=condt, in_=cond_d[:])
            lw = wp.tile([128, 4, 2 * C], F32R, name="lw")
            nc.gpsimd.dma_start(out=lw, in_=linw_d[:].rearrange("(j p) n -> p j n", p=128))
            gpool = wp.tile([128, 16], F32R, name="gpool")
            nc.gpsimd.dma_start(out=gpool, in_=gpool_d[:])
            gbcast = wp.tile([16, 128], F32R, name="gbcast")
            nc.gpsimd.dma_start(out=gbcast, in_=gbcast_d[:])
            linbT = wp.tile([128, 4], F32, name="linbT")
            nc.sync.dma_start(out=linbT, in_=linbT_d[:])
            qkvbT = wp.tile([128, 6], F32, name="qkvbT")
            nc.sync.dma_start(out=qkvbT, in_=qkvbT_d[:])
            projbT = wp.tile([128, 2], F32, name="projbT")
            nc.sync.dma_start(out=projbT, in_=projbT_d[:])
            ones8 = wp.tile([128, 2, 128], FP8, name="ones8")
            nc.vector.memset(ones8, 1.0)
            wqkv = wp.tile([128, CT, 3 * C], BF16, name="wqkv")
            nc.gpsimd.dma_start(out=wqkv, in_=qkvw_d[:].rearrange("(k p) n -> p k n", p=128))
            pw = wp.tile([128, CT, C], BF16, name="pw")
            nc.gpsimd.dma_start(out=pw, in_=projw_d[:].rearrange("(k p) n -> p k n", p=128))

            for _rep in range(reps):
              _ = _rep
              for t in range(CT):
                  for ch in range(4):
                      sl = slice(ch * 1024, (ch + 1) * 1024)
                      nc.sync.dma_start(out=xt[t][:, sl],
                                        in_=xt_d[t * 128:(t + 1) * 128, sl])
              # ================= Phase A: AdaGN scale/bias + GroupNorm stats ====
              # sb = cond @ lin_w  -> [1, 512] (PSUM)
              sb_ps = ps.tile([1, 2 * C], F32, tag="d", name="sb_ps")
              for j in range(4):
                  nc.tensor.matmul(sb_ps[0:1, :], condt[:, j:j + 1], lw[:, j, :],
                                   start=(j == 0), stop=(j == 3))
              # transpose to [128, 4] (cols: s_lo, s_hi, b_lo, b_hi) via strided DMA
              sb_sb = ss.tile([1, 2 * C], F32, name="sb_sb", bufs=1)
              nc.vector.tensor_copy(sb_sb, sb_ps)
              sbT = ss.tile([128, 4], F32, name="sbT")
              nc.sync.dma_start(out=sbsc_d[:].rearrange("j p -> () (j p)"), in_=sb_sb)
              nc.sync.dma_start(out=sbT, in_=sbsc_d[:].rearrange("j p -> p j"))
              sbv = ss.tile([128, 4], F32, name="sbv")
              nc.vector.tensor_add(sbv, sbT, linbT)

              eps16 = ss.tile([16, 1], F32, name="eps16")
              nc.vector.memset(eps16, EPS)

              AB = []  # per c-tile (A, B) [128,1] each
              for t in range(CT):
                  # per-channel mean/var over 4096 tokens
                  stats = ss.tile([128, 8, 6], F32, name=f"stats{t}")
                  for i in range(8):
                      nc.vector.bn_stats(out=stats[:, i, :],
                                         in_=xt[t][:, i * 512:(i + 1) * 512])
                  mv = ss.tile([128, 2], F32, name=f"mv{t}")
                  nc.vector.bn_aggr(out=mv, in_=stats)
                  # (mean, E[x^2]) per channel
                  st2 = ss.tile([128, 2], F32R, name=f"st2{t}")
                  nc.vector.tensor_copy(st2[:, 0:1], mv[:, 0:1])
                  nc.vector.tensor_tensor(st2[:, 1:2], mv[:, 0:1], mv[:, 0:1], op=OP.mult)
                  nc.vector.tensor_add(st2[:, 1:2], st2[:, 1:2], mv[:, 1:2])
                  # pool over groups of 8 channels (across partitions)
                  gst = ps.tile([16, 2], F32, tag="o", name=f"gst{t}", bufs=2)
                  nc.tensor.matmul(gst, gpool, st2, start=True, stop=True)
                  gm = ss.tile([16, 1], F32, name=f"gm{t}")
                  nc.vector.tensor_scalar_mul(gm, gst[:, 0:1], 1.0 / GS)
                  ge = ss.tile([16, 1], F32, name=f"ge{t}")
                  nc.vector.tensor_scalar_mul(ge, gst[:, 1:2], 1.0 / GS)
                  gv = ss.tile([16, 1], F32, name=f"gv{t}")
                  nc.vector.tensor_tensor(gv, gm, gm, op=OP.mult)
                  nc.vector.tensor_sub(gv, ge, gv)
                  # rstd = rsqrt(var + eps) via Newton on DVE (y0 = 1, 3 iters)
                  nc.vector.tensor_add(gv, gv, eps16)
                  ny = ss.tile([16, 1], F32, name=f"ny{t}")
                  nc.vector.memset(ny, 1.0)
                  nt = ss.tile([16, 1], F32, name=f"nt{t}")
                  for _it in range(3):
                      nc.vector.tensor_tensor(nt, ny, ny, op=OP.mult)
                      nc.vector.tensor_tensor(nt, gv, nt, op=OP.mult)
                      nc.vector.tensor_scalar(nt, nt, -0.5, 1.5, op0=OP.mult, op1=OP.add)
                      nc.vector.tensor_tensor(ny, ny, nt, op=OP.mult)
                  nc.vector.tensor_copy(gv, ny)
                  gvals = ss.tile([16, 2], F32R, name=f"gvals{t}")
                  nc.vector.tensor_copy(gvals[:, 0:1], gm)
                  nc.vector.tensor_copy(gvals[:, 1:2], gv)
                  # broadcast back to channels
                  chan = ps.tile([128, 2], F32, tag="o", name=f"chan{t}", bufs=2)
                  nc.tensor.matmul(chan, gbcast, gvals, start=True, stop=True)
                  # A = rstd*(1+scale); Bb = bias - mean*A
                  a_t = ss.tile([128, 1], F32, name=f"a{t}")
                  nc.vector.tensor_scalar_add(a_t, sbv[:, t:t + 1], 1.0)
                  nc.vector.tensor_tensor(a_t, a_t, chan[:, 1:2], op=OP.mult)
                  b_t = ss.tile([128, 1], F32, name=f"b{t}")
                  nc.vector.tensor_tensor(b_t, chan[:, 0:1], a_t, op=OP.mult)
                  nc.vector.tensor_sub(b_t, sbv[:, 2 + t:3 + t], b_t)
                  AB.append((a_t, b_t))

              # h = x*A + B (GPSIMD: SBUF-only elementwise)
              for ch in range(4):
                  sl = slice(ch * 1024, (ch + 1) * 1024)
                  for t in range(CT):
                      a_t, b_t = AB[t]
                      nc.gpsimd.tensor_scalar(out=hh[t][:, sl], in0=xt[t][:, sl],
                                              scalar1=a_t, scalar2=b_t,
                                              op0=OP.mult, op1=OP.add)

              h = hh  # normalized tokens, channel-major (bf16)

              # ================= Phase B: k/v/q projections =====================
              # k: all tokens, chunk order matches attention's kt consumption
              for c4 in range(4):
                  for m in range(CT):
                      kp = ps.tile([128, 1024], F32, tag="s", name="k_ps", bufs=2)
                      for j in range(2):
                          for ci in range(CT):
                              nc.tensor.matmul(
                                  kp[:, j * 512:(j + 1) * 512],
                                  wqkv[:, ci, C + m * 128: C + (m + 1) * 128],
                                  h[ci][:, c4 * 1024 + j * 512: c4 * 1024 + (j + 1) * 512],
                                  start=(ci == 0), stop=(ci == CT - 1))
                      nc.vector.tensor_scalar(
                          out=kT8[:, m, c4 * 1024:(c4 + 1) * 1024], in0=kp,
                          scalar1=qkvbT[:, 2 + m:3 + m], scalar2=0.25,
                          op0=OP.add, op1=OP.mult)
              # v: token-major (bias folded into proj bias on host)
              for tb in range(KT):
                  vp = ps.tile([128, C], F32, tag="o", name="v_ps", bufs=2)
                  for ci in range(CT):
                      nc.tensor.matmul(
                          vp, h[ci][:, tb * 128:(tb + 1) * 128],
                          wqkv[:, ci, 2 * C:3 * C],
                          start=(ci == 0), stop=(ci == CT - 1))
                  nc.vector.tensor_copy(vtok[:, tb, :], vp)
              # q: first TQ tokens only
              for half in range(2):
                  for m in range(CT):
                      qp = ps.tile([128, 1024], F32, tag="s", name="q_ps", bufs=2)
                      for j in range(2):
                          for ci in range(CT):
                              nc.tensor.matmul(
                                  qp[:, j * 512:(j + 1) * 512],
                                  wqkv[:, ci, m * 128:(m + 1) * 128],
                                  h[ci][:, half * 1024 + j * 512: half * 1024 + (j + 1) * 512],
                                  start=(ci == 0), stop=(ci == CT - 1))
                      nc.vector.tensor_scalar(
                          out=qT8[:, m, half * 1024:(half + 1) * 1024], in0=qp,
                          scalar1=qkvbT[:, m:m + 1], scalar2=0.25,
                          op0=OP.add, op1=OP.mult)

              # ================= Phase C: attention =============================
              for qc in range(NQC):
                  qsl = slice(qc * QC, (qc + 1) * QC)
                  o_ps = [ps.tile([128, QC], F32, tag="o", name=f"o_ps{t}", bufs=2)
                          for t in range(CT)]
                  den = ps.tile([128, QC], F32, tag="d", name="den")
                  for p in range(NP):
                      s2 = ps.tile([128, 2, QC], F32, tag="s", name="s2", bufs=2)
                      for i in range(2):
                          kt = 2 * p + i
                          nc.tensor.matmul(
                              s2[:, i, :],
                              kT8[:, :, kt * 128:(kt + 1) * 128],
                              qT8[:, :, qsl],
                              start=True, stop=True, perf_mode=DR)
                      p8 = sp.tile([128, 2, QC], FP8, tag="p", name="p8")
                      nc.scalar.activation(out=p8, in_=s2, func=AF.Exp)
                      for t in range(CT):
                          nc.tensor.matmul(
                              o_ps[t],
                              vtok[:, 2 * p:2 * p + 2, t * 128:(t + 1) * 128],
                              p8,
                              start=(p == 0), stop=(p == NP - 1),
                              perf_mode=DR)
                      nc.tensor.matmul(
                          den, ones8, p8,
                          start=(p == 0), stop=(p == NP - 1),
                          perf_mode=DR)
                  # softmax denominator (replicated on all partitions) -> recip
                  rb = rp.tile([128, QC], F32, tag="rb", name="rb")
                  nc.vector.reciprocal(rb, den)
                  for t in range(CT):
                      nc.vector.tensor_tensor(oT[t][:, qsl], o_ps[t], rb, op=OP.mult)
                  # proj + bias + residual for this q-chunk
                  for m in range(CT):
                      pj = ps.tile([128, QC], F32, tag="o", name="pj_ps", bufs=2)
                      for ci in range(CT):
                          nc.tensor.matmul(
                              pj,
                              pw[:, ci, m * 128:(m + 1) * 128],
                              oT[ci][:, qsl],
                              start=(ci == 0), stop=(ci == CT - 1))
                      fin = sw.tile([128, QC], F32, name="fin")
                      nc.vector.tensor_scalar_add(fin, pj, projbT[:, m:m + 1])
                      nc.vector.tensor_add(fin, fin, xt[m][:, qsl])
                      nc.sync.dma_start(
                          out=out_d[m * 128:(m + 1) * 128, qsl],
                          in_=fin)

    nc.compile()
    return nc


_GPOOL = np.zeros((128, 16), np.float32)
for _c in range(128):
    _GPOOL[_c, _c // GS] = 1.0
_GBCAST = np.ascontiguousarray(_GPOOL.T)

_NC_CACHE = None


def _get_nc():
    global _NC_CACHE
    if _NC_CACHE is None:
        _NC_CACHE = build_nc()
    return _NC_CACHE


def make_in_maps(x, cond, lin_w, lin_b, qkv_w, qkv_b, proj_w, proj_b):
    x = np.asarray(x, np.float32)
    cond = np.asarray(cond, np.float32)
    qkv_b = np.asarray(qkv_b, np.float32)
    proj_w = np.asarray(proj_w, np.float32)
    # v-bias contributes proj_w^T @ b_v to every output token; fold into proj_b
    pb_eff = np.asarray(proj_b, np.float32) + qkv_b[2 * C:3 * C] @ proj_w
    base = {
        "lin_w": np.ascontiguousarray(np.asarray(lin_w, np.float32)),
        "lin_bT": np.ascontiguousarray(np.asarray(lin_b, np.float32).reshape(4, 128).T),
        "qkv_w": np.ascontiguousarray(np.asarray(qkv_w, np.float32)),
        "qkv_bT": np.ascontiguousarray(qkv_b.reshape(6, 128).T),
        "proj_w": np.ascontiguousarray(proj_w),
        "proj_bT": np.ascontiguousarray(pb_eff.reshape(2, 128).T),
        "gpool": _GPOOL,
        "gbcast": _GBCAST,
    }
    in_maps = []
    for core in range(N_CORES):
        b, half = core // 2, core % 2
        x2 = x[b].reshape(C, HW)
        if half:
            x2 = np.concatenate([x2[:, TQ:], x2[:, :TQ]], axis=1)
        m = dict(base)
        m["xt"] = np.ascontiguousarray(x2)
        m["cond_t"] = np.ascontiguousarray(cond[b].reshape(4, 128).T)
        in_maps.append(m)
    return in_maps


def assemble(results):
    full = np.empty((B, C, HW), np.float32)
    for core in range(N_CORES):
        b, half = core // 2, core % 2
        full[b][:, half * TQ:(half + 1) * TQ] = results[core]["out"]
    return full.reshape(B, C, 64, 64)


def kernel(x, cond, lin_w, lin_b, qkv_w, qkv_b, proj_w, proj_b, **run_kwargs):
    nc = _get_nc()
    in_maps = make_in_maps(x, cond, lin_w, lin_b, qkv_w, qkv_b, proj_w, proj_b)
    res = run_bass_kernel_spmd(nc, in_maps, list(range(N_CORES)), **run_kwargs)
    out = assemble(res.results)
    if run_kwargs:
        kernel.last_result = res
    return out
